# revision 1
# baseline (speedup 1.0000x reference)
"""MultiHead HGNN attention (B=2, S=4096, D=256, H=4) on 8 TRN2 NeuronCores.

Sharding: query rows are split 8 ways (512 rows/core); every core computes all
batches/heads for its query block. The score matrix is built k-major
(scores^T) so the probs@V contraction needs no transposes; G^T is produced
on-device via a bf16 cast + xbar DMA transpose. Softmax denominators ride as
an extra ones-column in the V operand; normalization is applied to ctx^T via
a broadcast of 1/denom. Matmul operands are bf16 (fp32 streams at half rate
through the PE); accumulation stays fp32 in PSUM.
"""

import contextlib
import ctypes
import sys
import types

import numpy as np

sys.path.insert(0, "/opt/trn_rl_repo")


def _install_axon_hooks():
    """The agent image's antenv lacks axon_hooks; provide it so bass_utils can
    NTFF-profile under axon. Harmless when profiling is never requested."""
    if "antenv.axon_hooks" in sys.modules:
        return
    try:
        import antenv
    except ImportError:
        return
    mod = types.ModuleType("antenv.axon_hooks")
    holder = {}
    mod.set_axon_ntff_profile_hook = lambda h: holder.__setitem__("h", h)
    mod.get_axon_ntff_profile_hook = lambda: holder.get("h")
    sys.modules["antenv.axon_hooks"] = mod
    antenv.axon_hooks = mod
    try:
        lib = ctypes.CDLL("/opt/axon/libaxon_pjrt.so")
    except OSError:
        return
    if not hasattr(lib, "axon_start_nrt_profile"):
        return
    lib.axon_start_nrt_profile.argtypes = [ctypes.POINTER(ctypes.c_int64), ctypes.c_size_t]
    lib.axon_start_nrt_profile.restype = ctypes.c_int64
    lib.axon_stop_nrt_profile.argtypes = [ctypes.c_char_p]
    lib.axon_stop_nrt_profile.restype = ctypes.c_int64

    @contextlib.contextmanager
    def _hook(output_dir, device_ids):
        import jax

        jax.devices()
        if device_ids:
            ids = (ctypes.c_int64 * len(device_ids))(*device_ids)
            rc = lib.axon_start_nrt_profile(ids, len(device_ids))
        else:
            rc = lib.axon_start_nrt_profile(None, 0)
        if rc != 0:
            raise RuntimeError(f"axon_start_nrt_profile rc={rc}")
        try:
            yield
        finally:
            n = lib.axon_stop_nrt_profile(str(output_dir).encode())
            print(f"profile: {n} file(s) written to {output_dir}")

    mod.set_axon_ntff_profile_hook(_hook)


_install_axon_hooks()

B, S, D, H, HD = 2, 4096, 256, 4, 64
NCORES = 8
QR = S // NCORES          # 512 query rows per core
KC = S // 128             # 32 key chunks of 128
SCALE = 1.0 / np.sqrt(HD)

_BUILT = {}


def build_bass():
    if "nc" in _BUILT:
        return _BUILT["nc"]

    import concourse.tile as tile
    from concourse import bacc, mybir

    f32, bf16 = mybir.dt.float32, mybir.dt.bfloat16
    af = mybir.ActivationFunctionType

    nc = bacc.Bacc("TRN2", target_bir_lowering=False, debug=False, num_devices=NCORES)

    xt_in = nc.dram_tensor("xt", [B, 2, 128, S], bf16, kind="ExternalInput")
    xqt_in = nc.dram_tensor("xqt", [B, 2, 128, QR], bf16, kind="ExternalInput")
    g_in = nc.dram_tensor("g", [QR, S], bf16, kind="ExternalInput")
    wq_in = nc.dram_tensor("wq", [2, 128, 256], bf16, kind="ExternalInput")
    wk_in = nc.dram_tensor("wk", [2, 128, 256], bf16, kind="ExternalInput")
    wv_in = nc.dram_tensor("wv", [2, 128, 260], bf16, kind="ExternalInput")
    sel_in = nc.dram_tensor("sel", [1, 260], bf16, kind="ExternalInput")
    wo_in = nc.dram_tensor("wo", [H, 64, 256], f32, kind="ExternalInput")
    bias_in = nc.dram_tensor("bias", [1, 256], f32, kind="ExternalInput")
    out_dram = nc.dram_tensor("out", [B, QR, 256], f32, kind="ExternalOutput")

    with tile.TileContext(nc) as tc, contextlib.ExitStack() as ctx:
        cp = ctx.enter_context(tc.tile_pool(name="const", bufs=1))
        # 3 slots x 2 banks for scores / QKV staging / out-proj, + 2 ctx accumulators
        ps_big = ctx.enter_context(tc.tile_pool(name="ps_big", bufs=3, space="PSUM"))
        ps_ct = ctx.enter_context(tc.tile_pool(name="ps_ct", bufs=1, space="PSUM"))

        # ---- constants / weights ----
        wq_sb = cp.tile([128, 2, 256], bf16, tag="wq")
        wk_sb = cp.tile([128, 2, 256], bf16, tag="wk")
        wv_sb = cp.tile([128, 2, 260], bf16, tag="wv")
        sel_sb = cp.tile([1, 260], bf16, tag="sel")
        bias_sb = cp.tile([1, 256], f32, tag="bias")
        ones_sb = cp.tile([1, 128], f32, tag="ones")
        ones_bf = cp.tile([1, 128], bf16, tag="ones_bf")
        for ic in range(2):
            nc.sync.dma_start(wq_sb[:, ic, :], wq_in[ic])
            nc.sync.dma_start(wk_sb[:, ic, :], wk_in[ic])
            nc.sync.dma_start(wv_sb[:, ic, :], wv_in[ic])
        nc.sync.dma_start(sel_sb[:], sel_in[:])
        nc.sync.dma_start(bias_sb[:], bias_in[:])
        nc.gpsimd.memset(ones_sb[:], 1.0)
        nc.gpsimd.memset(ones_bf[:], 1.0)
        wo_sb = []
        for h in range(H):
            t = cp.tile([64, 256], f32, tag=f"wo{h}", name=f"wo{h}")
            nc.sync.dma_start(t[:], wo_in[h])
            wo_sb.append(t)

        gt_sb = cp.tile([128, KC, QR], bf16, tag="gt")
        # ---- G^T build: load f32 rows, cast to bf16 (DVE), xbar-transpose.
        # All transposes go through nc.scalar's queue, kc-major so early key
        # chunks of gt become consumable while later ones are still in flight.
        with tc.tile_pool(name="gbf", bufs=1) as pgb:
            gbfs = []
            for qc in range(QR // 128):
                gbf = pgb.tile([128, S], bf16, tag=f"gbf{qc}", name=f"gbf{qc}")
                nc.sync.dma_start(gbf[:], g_in[qc * 128:(qc + 1) * 128, :])
                gbfs.append(gbf)
            for kp in range(8):
                for qc in range(QR // 128):
                    nc.sync.dma_start_transpose(
                        gt_sb[:, kp * 4:(kp + 1) * 4, qc * 128:(qc + 1) * 128],
                        gbfs[qc][:, kp * 512:(kp + 1) * 512],
                    )


        # ---- x^T (host-pretransposed, bf16) ----
        xt_sb = [[cp.tile([128, S], bf16, tag=f"xt{b}{ic}", name=f"xt{b}{ic}") for ic in range(2)] for b in range(B)]
        for b in range(B):
            for ic in range(2):
                nc.sync.dma_start(xt_sb[b][ic][:], xt_in[b, ic])

        # ---- all-(b,hp) qT upfront (frees the xqt staging before main pools) ----
        qts = [[cp.tile([128, QR], bf16, tag=f"qt{b}{hp}", name=f"qt{b}{hp}") for hp in range(2)] for b in range(B)]
        with tc.tile_pool(name="xqp", bufs=1) as xqp:
            xqt_sb = xqp.tile([128, B, 2, QR], bf16, tag="xqt")
            for b in range(B):
                for ic in range(2):
                    nc.sync.dma_start(xqt_sb[:, b, ic, :], xqt_in[b, ic])
            for b in range(B):
                for hp in range(2):
                    aux = ps_big.tile([128, 2, 512], f32, tag="sc", name="auxq")
                    for ic in range(2):
                        nc.tensor.matmul(
                            aux[:, 0, :QR], wq_sb[:, ic, hp * 128:(hp + 1) * 128],
                            xqt_sb[:, b, ic, :], start=(ic == 0), stop=(ic == 1),
                        )
                    nc.vector.tensor_copy(qts[b][hp][:], aux[:, 0, :QR])

        ktp = ctx.enter_context(tc.tile_pool(name="ktp", bufs=2))
        vap = ctx.enter_context(tc.tile_pool(name="vap", bufs=2))
        ttp = ctx.enter_context(tc.tile_pool(name="ttp", bufs=3))
        pp = ctx.enter_context(tc.tile_pool(name="pp", bufs=4))
        otp = ctx.enter_context(tc.tile_pool(name="otp", bufs=2))
        rp = ctx.enter_context(tc.tile_pool(name="rp", bufs=2))

        ctf = [[None] * H for _ in range(B)]

        for b in range(B):
            for hp in range(2):
                qt = qts[b][hp]

                # ---- lazy K/V for this (batch, head-pair) ----
                kt = ktp.tile([128, S], bf16, tag="kt")
                for sc8 in range(8):
                    aux = ps_big.tile([128, 2, 512], f32, tag="sc", name="auxk")
                    for ic in range(2):
                        nc.tensor.matmul(
                            aux[:, 0, :], wk_sb[:, ic, hp * 128:(hp + 1) * 128],
                            xt_sb[b][ic][:, sc8 * 512:(sc8 + 1) * 512],
                            start=(ic == 0), stop=(ic == 1),
                        )
                    nc.scalar.copy(kt[:, sc8 * 512:(sc8 + 1) * 512], aux[:, 0, :])

                va = vap.tile([128, KC, 130], bf16, tag="va")
                for sg in range(KC // 2):
                    aux = ps_big.tile([128, 2, 512], f32, tag="sc", name="auxv")
                    for j in range(2):
                        kcj = sg * 2 + j
                        for ic in range(2):
                            nc.tensor.matmul(
                                aux[:, j, 0:130],
                                xt_sb[b][ic][:, kcj * 128:(kcj + 1) * 128],
                                wv_sb[:, ic, hp * 130:(hp + 1) * 130],
                                start=(ic == 0), stop=(ic == 1),
                            )
                    nc.scalar.copy(va[:, sg * 2:(sg + 1) * 2, :], aux[:, 0:2, 0:130])
                nc.vector.memset(va[:, :, 64:65], 1.0)
                nc.vector.memset(va[:, :, 129:130], 1.0)

                # ---- main loop over key chunks ----
                ct0 = ps_ct.tile([65, QR], f32, tag="ct0")
                ct1 = ps_ct.tile([65, QR], f32, tag="ct1")
                for kc in range(KC):
                    scp = ps_big.tile([128, 2, QR], f32, tag="sc", name="scp")
                    nc.tensor.matmul(
                        scp[:, 0, :], kt[0:64, kc * 128:(kc + 1) * 128], qt[0:64, :],
                        start=True, stop=True, tile_position=(0, 0),
                    )
                    nc.tensor.matmul(
                        scp[:, 1, :], kt[64:128, kc * 128:(kc + 1) * 128], qt[64:128, :],
                        start=True, stop=True, tile_position=(64, 0),
                    )
                    tt = ttp.tile([128, 2 * QR], f32, tag="tt")
                    nc.vector.tensor_mul(
                        tt[:, :].rearrange("p (a b) -> p a b", a=2), scp[:, :, :],
                        gt_sb[:, kc:kc + 1, :].broadcast_to([128, 2, QR]),
                    )
                    pt = pp.tile([128, 2, QR], bf16, tag="pt")
                    nc.scalar.activation(pt[:, :, :].rearrange("p a b -> p (a b)"), tt[:, :], af.Exp)
                    nc.tensor.matmul(
                        ct0[:, :], va[:, kc, 0:65], pt[:, 0, :],
                        start=(kc == 0), stop=(kc == KC - 1),
                    )
                    nc.tensor.matmul(
                        ct1[:, :], va[:, kc, 65:130], pt[:, 1, :],
                        start=(kc == 0), stop=(kc == KC - 1),
                    )

                # ---- stash unnormalized ctx^T; per-pair 1/denom dance ----
                c0 = cp.tile([64, QR], f32, tag=f"ctf{b}_{2 * hp}", name=f"ctf{b}_{2 * hp}")
                c1 = cp.tile([64, QR], f32, tag=f"ctf{b}_{2 * hp + 1}", name=f"ctf{b}_{2 * hp + 1}")
                nc.scalar.copy(c0[:], ct0[0:64, :])
                nc.scalar.copy(c1[:], ct1[0:64, :])
                den = rp.tile([1, 2, QR], f32, tag="den")
                nc.vector.tensor_copy(den[0:1, 0, :], ct0[64:65, :])
                nc.vector.tensor_copy(den[0:1, 1, :], ct1[64:65, :])
                denp = rp.tile([128, 8], f32, tag="denp")
                nc.sync.dma_start(denp[:, :], den[0:1, :, :])
                recp = rp.tile([128, 8], f32, tag="recp")
                nc.vector.reciprocal(recp[:], denp[:])
                rec = rp.tile([1, 2, QR], f32, tag="rec")
                nc.sync.dma_start(rec[0:1, :, :], recp[:, :])
                for j, cj in ((0, c0), (1, c1)):
                    bcb = rp.tile([64, QR], f32, tag="bcb")
                    nc.sync.dma_start(bcb[:, :], rec[0:1, j, :].rearrange("p (o q) -> p o q", o=1).broadcast_to([1, 64, QR]))
                    nc.vector.tensor_mul(cj[:], cj[:], bcb[:, :])
                ctf[b][2 * hp] = c0
                ctf[b][2 * hp + 1] = c1

            # ---- output projection for batch b ----
            for qs in range(QR // 128):
                op = ps_big.tile([128, 2, 512], f32, tag="sc", name="auxo")
                for h in range(H):
                    nc.tensor.matmul(
                        op[:, 0, 0:256], ctf[b][h][:, qs * 128:(qs + 1) * 128],
                        wo_sb[h][:], start=(h == 0), stop=False,
                    )
                nc.tensor.matmul(op[:, 0, 0:256], ones_sb[0:1, :], bias_sb[0:1, :], start=False, stop=True)
                ot = otp.tile([128, 256], f32, tag="ot")
                nc.vector.tensor_copy(ot[:], op[:, 0, 0:256])
                nc.sync.dma_start(out_dram[b, qs * 128:(qs + 1) * 128, :], ot[:])

    nc.compile()
    _BUILT["nc"] = nc
    return nc


def host_inputs(x, G, Wq, Wk, Wv, Wo, bo, b_extra):
    """Build the per-core input maps (layout prep + query-row sharding)."""
    import ml_dtypes

    f = np.float32
    bf = ml_dtypes.bfloat16
    x = np.asarray(x, f)
    G = np.asarray(G, f)
    xt = np.ascontiguousarray(x.transpose(0, 2, 1)).reshape(B, 2, 128, S).astype(bf)
    wq = np.ascontiguousarray(np.asarray(Wq, f).T * SCALE).reshape(2, 128, 256).astype(bf)
    wk = np.ascontiguousarray(np.asarray(Wk, f).T).reshape(2, 128, 256).astype(bf)
    wvt = np.asarray(Wv, f).T  # [256 in, 256 out]
    wv = np.zeros((2, 128, 260), f)
    for hp in range(2):
        wv[:, :, hp * 130:hp * 130 + 64] = wvt[:, hp * 128:hp * 128 + 64].reshape(2, 128, 64)
        wv[:, :, hp * 130 + 65:hp * 130 + 129] = wvt[:, hp * 128 + 64:hp * 128 + 128].reshape(2, 128, 64)
    wv = wv.astype(bf)
    sel = np.zeros((1, 260), f)
    sel[0, [64, 129, 194, 259]] = 1.0
    sel = sel.astype(bf)
    wo = np.ascontiguousarray(np.asarray(Wo, f).T).reshape(H, 64, 256)
    bias = (np.asarray(bo, f) + np.asarray(b_extra, f)).reshape(1, 256)

    shared = {"xt": xt, "wq": wq, "wk": wk, "wv": wv, "sel": sel, "wo": wo, "bias": bias}
    in_maps = []
    for c in range(NCORES):
        q0 = c * QR
        m = dict(shared)
        m["g"] = np.ascontiguousarray(G[q0:q0 + QR, :]).astype(bf)
        m["xqt"] = np.ascontiguousarray(xt[:, :, :, q0:q0 + QR])
        in_maps.append(m)
    return in_maps


def run(in_maps, trace=False):
    from concourse.bass_utils import run_bass_kernel_spmd

    nc = build_bass()
    return run_bass_kernel_spmd(nc, in_maps, core_ids=list(range(NCORES)), trace=trace)


def kernel(x, G, Wq, Wk, Wv, Wo, bo, b_extra):
    in_maps = host_inputs(x, G, Wq, Wk, Wv, Wo, bo, b_extra)
    res = run(in_maps, trace=False)
    out = np.concatenate([res.results[c]["out"] for c in range(NCORES)], axis=1)
    return out.astype(np.float32)



# revision 10
# speedup vs baseline: 1.1557x; 1.1557x over previous
"""MultiHead HGNN attention (B=2, S=4096, D=256, H=4) on 8 TRN2 NeuronCores.

Sharding: batch x query-rows (core = (b, 1024-query block)); each core computes
all 4 heads for its block. G is pre-transposed on host (k-major), so the
score-mask multiply is a dense FD=1024 DVE instruction with no broadcast
operand. Heads run as sequential kc-loops so each mask-mul covers one head's
full 1024-query row; exp is batched 4 key-chunks (FD=4096) per ACT
instruction. All PSUM evacuations run on the Scalar engine so the Vector
engine does (nearly) nothing but the mask multiplies. K/V/Q staging for the
second head-pair is interleaved into the first two head-loops' PE slack;
softmax-denominator reciprocal work is deferred past the last loop to keep
the DVE FIFO unblocked.
"""

import contextlib
import ctypes
import sys
import types

import numpy as np

sys.path.insert(0, "/opt/trn_rl_repo")


def _install_axon_hooks():
    """The agent image's antenv lacks axon_hooks; provide it so bass_utils can
    NTFF-profile under axon. Harmless when profiling is never requested."""
    if "antenv.axon_hooks" in sys.modules:
        return
    try:
        import antenv
    except ImportError:
        return
    mod = types.ModuleType("antenv.axon_hooks")
    holder = {}
    mod.set_axon_ntff_profile_hook = lambda h: holder.__setitem__("h", h)
    mod.get_axon_ntff_profile_hook = lambda: holder.get("h")
    sys.modules["antenv.axon_hooks"] = mod
    antenv.axon_hooks = mod
    try:
        lib = ctypes.CDLL("/opt/axon/libaxon_pjrt.so")
    except OSError:
        return
    if not hasattr(lib, "axon_start_nrt_profile"):
        return
    lib.axon_start_nrt_profile.argtypes = [ctypes.POINTER(ctypes.c_int64), ctypes.c_size_t]
    lib.axon_start_nrt_profile.restype = ctypes.c_int64
    lib.axon_stop_nrt_profile.argtypes = [ctypes.c_char_p]
    lib.axon_stop_nrt_profile.restype = ctypes.c_int64

    @contextlib.contextmanager
    def _hook(output_dir, device_ids):
        import jax

        jax.devices()
        if device_ids:
            ids = (ctypes.c_int64 * len(device_ids))(*device_ids)
            rc = lib.axon_start_nrt_profile(ids, len(device_ids))
        else:
            rc = lib.axon_start_nrt_profile(None, 0)
        if rc != 0:
            raise RuntimeError(f"axon_start_nrt_profile rc={rc}")
        try:
            yield
        finally:
            n = lib.axon_stop_nrt_profile(str(output_dir).encode())
            print(f"profile: {n} file(s) written to {output_dir}")

    mod.set_axon_ntff_profile_hook(_hook)


_install_axon_hooks()

B, S, D, H, HD = 2, 4096, 256, 4, 64
NCORES = 8
QB = 1024                 # query rows per core
QBLKS = S // QB           # 4 query blocks per batch
KC = S // 128             # 32 key chunks of 128
SCALE = 1.0 / np.sqrt(HD)

_BUILT = {}


def build_bass():
    if "nc" in _BUILT:
        return _BUILT["nc"]

    import concourse.tile as tile
    from concourse import bacc, mybir

    f32, bf16 = mybir.dt.float32, mybir.dt.bfloat16
    af = mybir.ActivationFunctionType

    nc = bacc.Bacc("TRN2", target_bir_lowering=False, debug=False, num_devices=NCORES)

    xt_in = nc.dram_tensor("xt", [2, 128, S], bf16, kind="ExternalInput")
    xq_in = nc.dram_tensor("xq", [2, 128, QB], bf16, kind="ExternalInput")
    g_in = nc.dram_tensor("g", [128, KC, QB], bf16, kind="ExternalInput")
    wq_in = nc.dram_tensor("wq", [2, 128, 256], bf16, kind="ExternalInput")
    wk_in = nc.dram_tensor("wk", [2, 128, 256], bf16, kind="ExternalInput")
    wv_in = nc.dram_tensor("wv", [2, 128, 260], bf16, kind="ExternalInput")
    wo_in = nc.dram_tensor("wo", [H, 64, 256], bf16, kind="ExternalInput")
    bias_in = nc.dram_tensor("bias", [1, 256], bf16, kind="ExternalInput")
    out_dram = nc.dram_tensor("out", [8, 128, 256], f32, kind="ExternalOutput")

    with tile.TileContext(nc) as tc, contextlib.ExitStack() as ctx, \
            nc.allow_low_precision(reason="bf16 pipeline; validated by rel-err check"):
        cp = ctx.enter_context(tc.tile_pool(name="const", bufs=1))
        # PSUM: scores 2buf x 2 banks + ct 2 banks + staging 2 banks = 8 banks
        ps = ctx.enter_context(tc.tile_pool(name="ps", bufs=2, space="PSUM"))
        ps_ct = ctx.enter_context(tc.tile_pool(name="ps_ct", bufs=1, space="PSUM"))
        ps_st = ctx.enter_context(tc.tile_pool(name="ps_st", bufs=1, space="PSUM"))

        # ---- constants / weights ----
        wq_sb = cp.tile([128, 2, 256], bf16, tag="wq")
        wk_sb = cp.tile([128, 2, 256], bf16, tag="wk")
        wv_sb = cp.tile([128, 2, 260], bf16, tag="wv")
        wo_sb = cp.tile([64, H, 256], bf16, tag="wo")
        bias_sb = cp.tile([1, 256], bf16, tag="bias")
        ones_bf = cp.tile([1, 128], bf16, tag="ones_bf")
        wu = cp.tile([128, 8], f32, tag="wu")
        wu2 = cp.tile([128, 8], f32, tag="wu2")
        # warmup: prime the exp table set on ACT before the hot loop
        nc.gpsimd.memset(wu[:], 0.0)
        nc.scalar.activation(wu2[:], wu[:], af.Exp)
        nc.gpsimd.memset(ones_bf[:], 1.0)
        for ic in range(2):
            nc.sync.dma_start(wq_sb[:, ic, :], wq_in[ic])
            nc.sync.dma_start(wk_sb[:, ic, :], wk_in[ic])
            nc.sync.dma_start(wv_sb[:, ic, :], wv_in[ic])
        for h in range(H):
            nc.sync.dma_start(wo_sb[:, h, :], wo_in[h])
        nc.sync.dma_start(bias_sb[:], bias_in[:])

        # ---- big inputs: x^T (this core's batch) and G^T (k-major) ----
        xt_sb = [cp.tile([128, S], bf16, tag=f"xt{ic}", name=f"xt{ic}") for ic in range(2)]
        xq_sb = cp.tile([128, 2, QB], bf16, tag="xq")
        for ic in range(2):
            nc.sync.dma_start(xt_sb[ic][:], xt_in[ic])
            nc.sync.dma_start(xq_sb[:, ic, :], xq_in[ic])
        gt_sb = cp.tile([128, KC, QB], bf16, tag="gt")
        for kp in range(8):
            nc.sync.dma_start(
                gt_sb[:, kp * 4:(kp + 1) * 4, :], g_in[:, kp * 4:(kp + 1) * 4, :]
            )

        # ---- persistent per-hp tensors ----
        kt = [cp.tile([128, S], bf16, tag=f"kt{hp}", name=f"kt{hp}") for hp in range(2)]
        va = [cp.tile([128, KC, 130], bf16, tag=f"va{hp}", name=f"va{hp}") for hp in range(2)]
        qt = [cp.tile([128, QB], bf16, tag=f"qt{hp}", name=f"qt{hp}") for hp in range(2)]
        ctf = [cp.tile([64, QB], bf16, tag=f"ctf{h}", name=f"ctf{h}") for h in range(H)]

        # ones (denominator) columns -- staging copies never touch cols 64/129
        for hp in range(2):
            nc.vector.memset(va[hp][:, :, 64:65], 1.0)
            nc.vector.memset(va[hp][:, :, 129:130], 1.0)

        # ---- staging emitters (each consumes one rotation of `pool`) ----
        def emit_q(pool, hp):
            aux = pool.tile([128, 2, 512], f32, tag="st", name=f"auxq{hp}")
            for qh in range(2):
                for ic in range(2):
                    nc.tensor.matmul(
                        aux[:, qh, :], wq_sb[:, ic, hp * 128:(hp + 1) * 128],
                        xq_sb[:, ic, qh * 512:(qh + 1) * 512],
                        start=(ic == 0), stop=(ic == 1),
                    )
            nc.scalar.copy(qt[hp][:, :].rearrange("p (a b) -> p a b", a=2), aux[:, :, :])

        def emit_k(pool, hp, i):
            aux = pool.tile([128, 2, 512], f32, tag="st", name=f"auxk{hp}_{i}")
            for j in range(2):
                for ic in range(2):
                    nc.tensor.matmul(
                        aux[:, j, :], wk_sb[:, ic, hp * 128:(hp + 1) * 128],
                        xt_sb[ic][:, (2 * i + j) * 512:(2 * i + j + 1) * 512],
                        start=(ic == 0), stop=(ic == 1),
                    )
            nc.scalar.copy(
                kt[hp][:, i * 1024:(i + 1) * 1024].rearrange("p (a b) -> p a b", a=2),
                aux[:, :, :],
            )

        def emit_v(pool, sg):
            # V for BOTH hp at once (wv columns 0:130 = hp0, 130:260 = hp1).
            # Copies skip the ones-columns (64/129) via a 65-strided view.
            aux = pool.tile([128, 2, 512], f32, tag="st", name=f"auxv{sg}")
            for j in range(2):
                kcj = sg * 2 + j
                for ic in range(2):
                    nc.tensor.matmul(
                        aux[:, j, 0:260],
                        xt_sb[ic][:, kcj * 128:(kcj + 1) * 128],
                        wv_sb[:, ic, :],
                        start=(ic == 0), stop=(ic == 1),
                    )
            for hp in range(2):
                src = aux[:, 0:2, hp * 130:(hp + 1) * 130].rearrange(
                    "p a (g d) -> p a g d", d=65
                )[:, :, :, 0:64]
                dst = va[hp][:, sg * 2:(sg + 1) * 2, :].rearrange(
                    "p a (g d) -> p a g d", d=65
                )[:, :, :, 0:64]
                nc.scalar.copy(dst, src)

        # ---- prep for hp0's first head: Q0, K0, V sg0..3 (kc 0..7) ----
        emit_q(ps, 0)
        for i in range(4):
            emit_k(ps, 0, i)
        for sg in range(4):
            emit_v(ps, sg)

        # deferred staging work, interleaved into the first two head-loops
        deferred = [("v", sg) for sg in range(4, 16)] + [("q", 1)] + [
            ("k", 1, i) for i in range(4)
        ]
        ndef = len(deferred)
        dpos = 0

        def drain_deferred(want):
            nonlocal dpos
            while dpos < min(want, ndef):
                d = deferred[dpos]
                if d[0] == "v":
                    emit_v(ps_st, d[1])
                elif d[0] == "q":
                    emit_q(ps_st, d[1])
                else:
                    emit_k(ps_st, d[1], d[2])
                dpos += 1

        ttp = ctx.enter_context(tc.tile_pool(name="ttp", bufs=2))
        ptp = ctx.enter_context(tc.tile_pool(name="ptp", bufs=2))
        rp = ctx.enter_context(tc.tile_pool(name="rp", bufs=2))
        otp = ctx.enter_context(tc.tile_pool(name="otp", bufs=2))

        # denominators for all 4 heads, spread across partitions (tiny)
        denP = cp.tile([128, 4, 8], bf16, tag="denP")
        cS = [cp.tile([64, QB], bf16, tag=f"cS{h}", name=f"cS{h}") for h in range(H)]

        for hp in range(2):
            for j in range(2):
                h = 2 * hp + j
                ct = ps_ct.tile([65, QB], f32, tag="ct", name=f"ct{h}")
                for kg in range(8):   # groups of 4 kc
                    # keep deferred staging 2 groups ahead of ctx consumption
                    if h == 0:
                        drain_deferred(min(12, 2 * kg + 4))
                    elif h == 1:
                        drain_deferred(12 + kg + 1)
                    tt = ttp.tile([128, 4, QB], bf16, tag="tt")
                    for ki in range(4):
                        kc = kg * 4 + ki
                        scp = ps.tile([128, 2, 512], f32, tag="st", name="scp")
                        for qh in range(2):
                            nc.tensor.matmul(
                                scp[:, qh, :],
                                kt[hp][j * 64:(j + 1) * 64, kc * 128:(kc + 1) * 128],
                                qt[hp][j * 64:(j + 1) * 64, qh * 512:(qh + 1) * 512],
                                start=True, stop=True, tile_position=(j * 64, 0),
                            )
                        nc.vector.tensor_mul(
                            tt[:, ki, :],
                            scp[:, :, :].rearrange("p a b -> p (a b)"),
                            gt_sb[:, kc, :],
                        )
                    pt = ptp.tile([128, 4, QB], bf16, tag="pt")
                    nc.scalar.activation(
                        pt[:, :, :].rearrange("p a b -> p (a b)"),
                        tt[:, :, :].rearrange("p a b -> p (a b)"),
                        af.Exp,
                    )
                    for ki in range(4):
                        kc = kg * 4 + ki
                        for qh in range(2):
                            nc.tensor.matmul(
                                ct[:, qh * 512:(qh + 1) * 512],
                                va[hp][:, kc, j * 65:(j + 1) * 65],
                                pt[:, ki, qh * 512:(qh + 1) * 512],
                                start=(kc == 0), stop=(kc == KC - 1),
                            )

                # evacuate ct promptly (frees the bank pair for the next head);
                # defer all DVE-side reciprocal work past the last loop
                denS = rp.tile([1, QB], bf16, tag="den", name=f"den{h}")
                nc.scalar.copy(denS[:], ct[64:65, :])
                nc.scalar.copy(cS[h][:], ct[0:64, :])
                nc.sync.dma_start(denP[:, h, :], denS[0:1, :])

        # ---- deferred normalization (reciprocal + broadcast multiply) ----
        recP = rp.tile([128, 4, 8], bf16, tag="recP")
        nc.vector.reciprocal(recP[:], denP[:])
        for h in range(H):
            recS = rp.tile([1, QB], bf16, tag="recS", name=f"recS{h}")
            nc.sync.dma_start(recS[0:1, :], recP[:, h, :])
            bcb = rp.tile([64, QB], bf16, tag="bcb", name=f"bcb{h}")
            nc.sync.dma_start(
                bcb[:, :],
                recS[0:1, :].rearrange("p (o q) -> p o q", o=1).broadcast_to([1, 64, QB]),
            )
            nc.vector.tensor_mul(ctf[h][:], cS[h][:], bcb[:, :])

        # ---- output projection ----
        for qs in range(8):
            op = ps_st.tile([128, 2, 512], f32, tag="st", name="auxo")
            for h in range(H):
                nc.tensor.matmul(
                    op[:, 0, 0:256], ctf[h][:, qs * 128:(qs + 1) * 128],
                    wo_sb[:, h, :], start=(h == 0), stop=False,
                )
            nc.tensor.matmul(op[:, 0, 0:256], ones_bf[0:1, :], bias_sb[0:1, :], start=False, stop=True)
            ot = otp.tile([128, 256], f32, tag="ot")
            nc.scalar.copy(ot[:], op[:, 0, 0:256])
            nc.sync.dma_start(out_dram[qs], ot[:])

    nc.compile()
    _BUILT["nc"] = nc
    return nc


def host_inputs(x, G, Wq, Wk, Wv, Wo, bo, b_extra):
    """Build the per-core input maps (layout prep + batch x query sharding)."""
    import ml_dtypes

    f = np.float32
    bf = ml_dtypes.bfloat16
    x = np.asarray(x, f)
    G = np.asarray(G, f)
    # per-batch x^T: [B, 2, 128, S]
    xt = np.ascontiguousarray(x.transpose(0, 2, 1)).reshape(B, 2, 128, S).astype(bf)
    wq = np.ascontiguousarray(np.asarray(Wq, f).T * SCALE).reshape(2, 128, 256).astype(bf)
    wk = np.ascontiguousarray(np.asarray(Wk, f).T).reshape(2, 128, 256).astype(bf)
    wvt = np.asarray(Wv, f).T  # [256 in, 256 out]
    wv = np.zeros((2, 128, 260), f)
    for hp in range(2):
        wv[:, :, hp * 130:hp * 130 + 64] = wvt[:, hp * 128:hp * 128 + 64].reshape(2, 128, 64)
        wv[:, :, hp * 130 + 65:hp * 130 + 129] = wvt[:, hp * 128 + 64:hp * 128 + 128].reshape(2, 128, 64)
    wv = wv.astype(bf)
    wo = np.ascontiguousarray(np.asarray(Wo, f).T).reshape(H, 64, 256).astype(bf)
    bias = (np.asarray(bo, f) + np.asarray(b_extra, f)).reshape(1, 256).astype(bf)
    GT = np.ascontiguousarray(G.T)  # [k, q]

    shared = {"wq": wq, "wk": wk, "wv": wv, "wo": wo, "bias": bias}
    in_maps = []
    for c in range(NCORES):
        b, qblk = c // QBLKS, c % QBLKS
        q0 = qblk * QB
        m = dict(shared)
        m["xt"] = xt[b]
        m["xq"] = np.ascontiguousarray(xt[b][:, :, q0:q0 + QB])
        m["g"] = np.ascontiguousarray(
            GT[:, q0:q0 + QB].reshape(KC, 128, QB).transpose(1, 0, 2)
        ).astype(bf)
        in_maps.append(m)
    return in_maps


def run(in_maps, trace=False):
    from concourse.bass_utils import run_bass_kernel_spmd

    nc = build_bass()
    return run_bass_kernel_spmd(nc, in_maps, core_ids=list(range(NCORES)), trace=trace)


def assemble_out(res):
    out = np.zeros((B, S, D), np.float32)
    for c in range(NCORES):
        b, qblk = c // QBLKS, c % QBLKS
        out[b, qblk * QB:(qblk + 1) * QB, :] = (
            np.asarray(res.results[c]["out"]).reshape(QB, D)
        )
    return out


def kernel(x, G, Wq, Wk, Wv, Wo, bo, b_extra):
    in_maps = host_inputs(x, G, Wq, Wk, Wv, Wo, bo, b_extra)
    res = run(in_maps, trace=False)
    return assemble_out(res)
